# revision 1
# baseline (speedup 1.0000x reference)
"""Trainium2 Bass kernel for an LSTM encoder-decoder chatbot model.

Model: question -> embed -> LSTM(512) -> linear(256) = q_out
       answer[:, :256] -> embed -> concat(q_out) -> LSTM(512) -> linear(32000)
Output: logits [B=32, W=32000, STEPS=256] f32.

Sharding: all 8 cores run the full (replicated) encoder + decoder
recurrence; the dominant 512x32000 output projection is sharded
column-wise (vocab) across cores; each core emits [32, 4000, 256].

Matmul strategy: hidden state kept transposed (hT fp16 [128, 4x32])
as the PE stationary operand; weights stream as the moving operand in
fp16. Four col-tiled matmuls (tile_position=(0,32c)) run concurrently,
one per 512-unit gate block, so the gates land on all 128 PSUM
partitions [(block,b), 512=i|f|g|o] and the elementwise LSTM cell runs
full-width. Gate columns are host-permuted accordingly.
"""
import sys
import numpy as np

sys.path.insert(0, '/opt/trn_rl_repo')

import concourse.bass as bass  # noqa: E402
import concourse.bacc as bacc  # noqa: E402
import concourse.mybir as mybir  # noqa: E402
import concourse.tile as tile  # noqa: E402
from concourse.bass import IndirectOffsetOnAxis  # noqa: E402
from concourse.bass_utils import run_bass_kernel_spmd  # noqa: E402

F32 = mybir.dt.float32
F16 = mybir.dt.float16
F8 = mybir.dt.float8e4
I32DT = mybir.dt.int32
AF = mybir.ActivationFunctionType
DR = mybir.MatmulPerfMode.DoubleRow

W_VOCAB = 32000
EMB = 256
STEPS = 256
HID = 512
QOUT = 256
B = 32
LQ = 50
NCORES = 8
VSH = W_VOCAB // NCORES      # 4000 vocab rows per core
VPAD = 4096                   # padded to 32 tiles of 128
G = 4 * HID                   # 2048 gate columns
TBLK = 32                     # decoder steps per hs block (8 blocks)

_cache = {}


def _gate_perm():
    """Block layout [i|f|o|g]x128 per 128-unit block: new col
    j = 512*blk + 128*slot + u  <-  old row 512*gate + 128*blk + u,
    with slot order (i, f, o, g) so the three sigmoids are contiguous."""
    j = np.arange(G)
    blk, r = j // 512, j % 512
    slot, u = r // 128, r % 128
    old_gate = np.array([0, 1, 3, 2])[slot]
    return 512 * old_gate + 128 * blk + u


def build_program():
    nc = bacc.Bacc("TRN2", target_bir_lowering=False, debug=False,
                   num_devices=NCORES)

    def inp(name, shape, dt):
        return nc.dram_tensor(name, shape, dt, kind="ExternalInput").ap()

    q_idx = inp("q_idx", [13 * 128], I32DT)            # padded 1664
    a_idx = inp("a_idx", [STEPS * B], I32DT)           # 8192, t-major
    q_emb = inp("q_emb", [W_VOCAB, EMB], F16)
    a_emb = inp("a_emb", [W_VOCAB, EMB], F16)
    w_ihT_enc = inp("w_ihT_enc", [EMB, G], F16)        # permuted cols
    w_hhT_enc = inp("w_hhT_enc", [HID, G], F16)
    bias_enc = inp("bias_enc", [1, G], F16)
    w_ihAT = inp("w_ihAT", [EMB, G], F16)
    w_ihQT = inp("w_ihQT", [QOUT, G], F16)
    w_hhT_dec = inp("w_hhT_dec", [HID, G], F16)
    bias_dec = inp("bias_dec", [1, G], F16)
    q_lin_wT = inp("q_lin_wT", [HID, QOUT], F16)
    q_lin_b = inp("q_lin_b", [1, QOUT], F16)
    lin_w8 = inp("lin_w8", [128, 2 * 2 * VPAD], F8)    # fp8 pairs, x16
    lin_b = inp("lin_b", [128, 32], F32)               # [u, mtile]
    i128f = inp("i128f", [128, 128], F32)
    i128h = inp("i128h", [128, 128], F16)
    i128s = inp("i128s", [128, 128], F16)              # 16*eye
    i32h = inp("i32h", [32, 32], F16)
    ones1 = inp("ones1", [1, 32], F16)
    out = nc.dram_tensor("out", [B, VPAD, STEPS], F32,
                         kind="ExternalOutput").ap()

    with tile.TileContext(nc) as tc:
        _build(nc, tc, locals())
    nc.compile()
    return nc


def _build(nc, tc, t):
    from contextlib import ExitStack
    ctx = ExitStack()
    with ctx:
        _build_inner(nc, tc, t, ctx)


def _build_inner(nc, tc, t, ctx):
    # ---- pools -------------------------------------------------------
    wpool = ctx.enter_context(tc.tile_pool(name="weights", bufs=1))
    const = ctx.enter_context(tc.tile_pool(name="const", bufs=1))
    embp = ctx.enter_context(tc.tile_pool(name="embp", bufs=6))
    seqp = ctx.enter_context(tc.tile_pool(name="seqp", bufs=1))
    state = ctx.enter_context(tc.tile_pool(name="state", bufs=4))
    ew = ctx.enter_context(tc.tile_pool(name="ew", bufs=4))
    hsp = ctx.enter_context(tc.tile_pool(name="hsp", bufs=3))
    outp = ctx.enter_context(tc.tile_pool(name="outp", bufs=8))
    ps_g = ctx.enter_context(tc.tile_pool(name="ps_g", bufs=3, space="PSUM"))
    ps_tr = ctx.enter_context(tc.tile_pool(name="ps_tr", bufs=2, space="PSUM"))
    ps_p = ctx.enter_context(tc.tile_pool(name="ps_p", bufs=2, space="PSUM"))

    def load(pool, ap, dt=None, name=None):
        s = pool.tile(list(ap.shape), dt or ap.dtype, tag=name, name=name or 'ld')
        nc.sync.dma_start(s[:], ap[:])
        return s

    def loadc(pool, ap, name):
        p, cdim = ap.shape
        n = p // 128
        s = pool.tile([128, n * cdim], ap.dtype, tag=name, name=name)
        for k in range(n):
            nc.sync.dma_start(s[:, cdim * k:cdim * (k + 1)],
                              ap[128 * k:128 * (k + 1), :])
        def chunk(k, sl=slice(None)):
            base = cdim * k
            if sl == slice(None):
                return s[:, base:base + cdim]
            return s[:, base + sl.start:base + sl.stop]
        return chunk

    # ---- resident weights/constants ---------------------------------
    wih_e = loadc(wpool, t["w_ihT_enc"], "wih_e")     # 2 chunks [128,2048]
    whh_e = loadc(wpool, t["w_hhT_enc"], "whh_e")     # 4 chunks
    b_e = load(const, t["bias_enc"], name="b_e")
    wihA = loadc(wpool, t["w_ihAT"], "wihA")
    wihQ = loadc(wpool, t["w_ihQT"], "wihQ")
    whh_d = loadc(wpool, t["w_hhT_dec"], "whh_d")
    b_d = load(const, t["bias_dec"], name="b_d")
    qlw = loadc(wpool, t["q_lin_wT"], "qlw")          # 4 chunks [128,256]
    qlb = load(const, t["q_lin_b"], name="qlb")
    linw8 = wpool.tile([128, 2, 2, VPAD], F8, tag="linw8", name="linw8")
    nc.sync.dma_start(linw8[:].rearrange("p a b c -> p (a b c)"), t["lin_w8"][:])
    linb = load(const, t["lin_b"], name="linb")           # [128, 32] f32
    I128f = load(const, t["i128f"], name="I128f")
    I128h = load(const, t["i128h"], name="I128h")
    I128s = load(const, t["i128s"], name="I128s")
    I32h = load(const, t["i32h"], name="I32h")
    ones = load(const, t["ones1"], name="ones")

    # index tiles
    qidx_sb = load(const, t["q_idx"].rearrange("(n p) -> n p", p=128)
                   .rearrange("n p -> p n"), name="qidx")   # [128, 13]
    aidx_sb = load(const, t["a_idx"].rearrange("(n p) -> n p", p=128)
                   .rearrange("n p -> p n"), name="aidx")   # [128, 64]

    # ---- embedding gather + transpose -> xT tiles --------------------
    def embed_T(table, idx_sb, ntiles, name):
        """gather rows (t-major) and transpose into xT [2 x [128, ntiles*128]] f16"""
        xT = [seqp.tile([128, ntiles * 128], F16, tag=f"{name}{k}", name=f"{name}{k}")
              for k in range(2)]
        for i in range(ntiles):
            rows = embp.tile([128, EMB], F16, tag="gather")
            nc.gpsimd.indirect_dma_start(
                out=rows[:], out_offset=None, in_=table[:],
                in_offset=IndirectOffsetOnAxis(ap=idx_sb[:, i:i + 1], axis=0))
            for k in range(2):
                p = ps_tr.tile([128, 128], F16, space="PSUM", tag="tr",
                               name="trp")
                nc.tensor.transpose(p[:], rows[:, 128 * k:128 * (k + 1)],
                                    I128h[:])
                nc.vector.tensor_copy(xT[k][:, 128 * i:128 * (i + 1)], p[:])
        return xT

    qT = embed_T(t["q_emb"], qidx_sb, 13, "qT")    # [256, 1664] f16
    # aT emission is deferred: tiles created now, per-tile gather+transpose
    # emitted interleaved into the encoder steps to fill PE chain gaps.
    aT = [seqp.tile([128, 64 * 128], F16, tag=f"aT{k}", name=f"aT{k}")
          for k in range(2)]

    def emit_aT(i):
        rows = embp.tile([128, EMB], F16, tag="gather", name="arows")
        nc.gpsimd.indirect_dma_start(
            out=rows[:], out_offset=None, in_=t["a_emb"][:],
            in_offset=IndirectOffsetOnAxis(ap=aidx_sb[:, i:i + 1], axis=0))
        for k in range(2):
            p = ps_tr.tile([128, 128], F16, space="PSUM", tag="tr",
                           name="trpa")
            nc.tensor.transpose(p[:], rows[:, 128 * k:128 * (k + 1)],
                                I128h[:])
            nc.vector.tensor_copy(aT[k][:, 128 * i:128 * (i + 1)], p[:])

    # ---- LSTM cell ---------------------------------------------------
    def step(hT, c_prev, seeds, wx_list, whh, has_h, want_hs):
        """One LSTM step, full-width col-tiled.

        seeds: list of (lhsT_ap[K,32], rhs_ap[K, 2048]) matmuls
        wx_list: list of (lhsT_ap, chunk_fn, k); whh: chunk accessor.
        """
        gp = ps_g.tile([128, 512], F32, space="PSUM", tag="gates")
        # rows = (lhsT_fn(sl), rhs_fn(sl)) emitted strip-innermost so the 4
        # col-strips run concurrently on the PE array
        rows = []
        for lhsT, rhs in seeds:
            rows.append((lambda sl, l=lhsT, r=rhs: (l, r[:, sl])))
        for lhsT, cf, k in wx_list:
            rows.append((lambda sl, l=lhsT, c2=cf, kk=k: (l, c2(kk, sl))))
        if has_h:
            for k in range(4):
                rows.append((lambda sl, kk=k: (hT[:, 32 * kk:32 * (kk + 1)],
                                               whh(kk, sl))))
        nrows = len(rows)
        for i, rowf in enumerate(rows):
            for c in range(4):
                sl = slice(512 * c, 512 * (c + 1))
                lhsT, rhs = rowf(sl)
                nc.tensor.matmul(gp[32 * c:32 * (c + 1), :], lhsT, rhs,
                                 start=(i == 0), stop=(i == nrows - 1),
                                 tile_position=(0, 32 * c))
        sig = ew.tile([128, 384], F32, tag="sig")   # i | f | o
        nc.scalar.activation(sig[:], gp[:, 0:384], AF.Sigmoid)
        gg = ew.tile([128, 128], F32, tag="g")
        nc.scalar.activation(gg[:], gp[:, 384:512], AF.Tanh)
        igg = ew.tile([128, 128], F32, tag="ig")
        nc.vector.tensor_mul(igg[:], sig[:, 0:128], gg[:])
        c_new = state.tile([128, 128], F32, tag="c")
        if c_prev is None:
            nc.vector.tensor_copy(c_new[:], igg[:])  # c0 = 0 -> c = i*g
        else:
            fc = ew.tile([128, 128], F32, tag="fc")
            nc.vector.tensor_mul(fc[:], sig[:, 128:256], c_prev[:])
            nc.vector.tensor_add(c_new[:], igg[:], fc[:])
        th = ew.tile([128, 128], F32, tag="th")
        nc.scalar.activation(th[:], c_new[:], AF.Tanh)
        h_new = ew.tile([128, 128], F16, tag="h")
        nc.vector.tensor_mul(h_new[:], sig[:, 256:384], th[:])
        trp = ps_tr.tile([128, 128], F16, space="PSUM", tag="tr", name="trh")
        nc.tensor.transpose(trp[:], h_new[:], I128h[:])
        hT_new = state.tile([128, 128], F16, tag="hT")
        nc.vector.tensor_copy(hT_new[:], trp[:])
        hT8_new = None
        if want_hs:
            trs = ps_tr.tile([128, 128], F32, space="PSUM", tag="trs",
                             name="trs", bufs=1)
            nc.tensor.matmul(trs[:], h_new[:], I128s[:], start=True, stop=True)
            hT8_new = state.tile([128, 128], F8, tag="hT8")
            nc.vector.tensor_copy(hT8_new[:], trs[:])
        return hT_new, hT8_new, c_new

    # ---- encoder -----------------------------------------------------
    hT = None
    c = None
    a_emitted = 0
    for tt in range(LQ):
        sl32 = slice(32 * tt, 32 * (tt + 1))
        seeds = [(ones[:], b_e[:])]
        wx = [(qT[0][:, sl32], wih_e, 0),
              (qT[1][:, sl32], wih_e, 1)]
        hT, _, c = step(hT, c, seeds, wx, whh_e, has_h=(tt > 0), want_hs=False)
        want = (tt + 1) * 64 // LQ
        while a_emitted < want:
            emit_aT(a_emitted)
            a_emitted += 1
    while a_emitted < 64:
        emit_aT(a_emitted)
        a_emitted += 1

    # ---- q_out = h @ q_lin_w.T + b; then Qb = q_out @ w_ihQ.T + bias_dec
    qo_p_t = ps_p.tile([128, 512], F32, space="PSUM", tag="proj", name="qo_p")
    qo_p = qo_p_t[0:32, 0:QOUT]
    nc.tensor.matmul(qo_p[:], ones[:], qlb[:], start=True, stop=False)
    for k in range(4):
        nc.tensor.matmul(qo_p[:], hT[:, 32 * k:32 * (k + 1)],
                         qlw(k), start=False, stop=(k == 3))
    qo = seqp.tile([32, QOUT], F16, tag="qo_sb")
    nc.scalar.activation(qo[:], qo_p[:], AF.Identity)
    # transpose q_out [32,256] -> [256(2x128), 32] f16
    qoT = seqp.tile([128, 64], F16, tag="qoT")
    for k in range(2):
        p = ps_tr.tile([128, 128], F16, space="PSUM", tag="tr", name="trq")
        nc.tensor.transpose(p[:, 0:32], qo[:, 128 * k:128 * (k + 1)], I32h[:])
        nc.vector.tensor_copy(qoT[:, 32 * k:32 * (k + 1)], p[:, 0:32])
    # Qb [32, 2048] f16, quarter at a time (no col tiling, partition 0-31)
    qb = seqp.tile([32, G], F16, tag="qb")
    for qtr in range(4):
        sl = slice(512 * qtr, 512 * (qtr + 1))
        qp = ps_p.tile([128, 512], F32, space="PSUM", tag="proj", name="qp")[0:32, :]
        nc.tensor.matmul(qp[:], ones[:], b_d[:, sl], start=True, stop=False)
        for k in range(2):
            nc.tensor.matmul(qp[:], qoT[:, 32 * k:32 * (k + 1)],
                             wihQ(k, sl), start=False, stop=(k == 1))
        nc.scalar.activation(qb[:, sl], qp[:], AF.Identity)

    # ---- decoder + projection, software-pipelined --------------------
    # Block b's 32 vocab-tile projections are emitted one per step during
    # block b+1's recurrence, filling PE gaps in the chain-bound LSTM.
    out = t["out"]

    def proj_m(hs, blk, m):
        hsk = hs.rearrange("p (k bt) -> p k bt", k=4)
        for s in range(TBLK * 32 // 512):
            pp = ps_p.tile([128, 512], F32, space="PSUM", tag="proj")
            for j in range(2):
                nc.tensor.matmul(
                    pp[:], linw8[:, j, :, 128 * m:128 * (m + 1)],
                    hsk[:, 2 * j:2 * j + 2, 512 * s:512 * (s + 1)],
                    start=(j == 0), stop=(j == 1), perf_mode=DR)
            ot = outp.tile([128, 512], F32, tag="ot")
            nc.scalar.activation(ot[:], pp[:], AF.Identity,
                                 scale=1.0 / 256.0, bias=linb[:, m:m + 1])
            nb = 512 // TBLK  # batches per sub-block
            dst = out[nb * s:nb * (s + 1), 128 * m:128 * (m + 1),
                      blk * TBLK:(blk + 1) * TBLK].rearrange("b w t -> w b t")
            nc.sync.dma_start(dst, ot[:].rearrange("w (b t) -> w b t", b=nb))

    hs_prev = None
    for blk in range(STEPS // TBLK):
        hs = hsp.tile([128, 4 * TBLK * 32], F8, tag="hs", name="hs")
        for dt in range(TBLK):
            tt = blk * TBLK + dt
            sl32 = slice(32 * tt, 32 * (tt + 1))
            seeds = [(I32h[:], qb[:])]
            wx = [(aT[0][:, sl32], wihA, 0),
                  (aT[1][:, sl32], wihA, 1)]
            hT, hT8, c = step(hT, c, seeds, wx, whh_d, has_h=True, want_hs=True)
            # scatter hT into the hs block: col (k*32*TBLK + b*TBLK + dt)
            dst = hs.rearrange("p (k b t) -> p k b t", k=4, b=32)[:, :, :, dt]
            nc.gpsimd.tensor_copy(dst, hT8[:].rearrange("p (k b) -> p k b", k=4))
            if hs_prev is not None:
                proj_m(hs_prev, blk - 1, dt)
        hs_prev = hs
    for m in range(VPAD // 128):
        proj_m(hs_prev, STEPS // TBLK - 1, m)


def kernel(**inputs):
    import ml_dtypes
    E4 = ml_dtypes.float8_e4m3
    inp = {k: np.asarray(v) for k, v in inputs.items()}
    if "prog" not in _cache:
        _cache["prog"] = build_program()
    nc = _cache["prog"]

    perm = _gate_perm()
    f16 = np.float16

    def prep_lstm(w_ih, w_hh, b_ih, b_hh):
        wihT = np.ascontiguousarray(w_ih.T[:, perm]).astype(f16)
        whhT = np.ascontiguousarray(w_hh.T[:, perm]).astype(f16)
        bias = (b_ih + b_hh)[perm][None, :].astype(f16)
        return wihT, whhT, bias

    wihT_e, whhT_e, b_e = prep_lstm(inp["q_lstm_w_ih"], inp["q_lstm_w_hh"],
                                    inp["q_lstm_b_ih"], inp["q_lstm_b_hh"])
    wihT_d, whhT_d, b_d = prep_lstm(inp["a_lstm_w_ih"], inp["a_lstm_w_hh"],
                                    inp["a_lstm_b_ih"], inp["a_lstm_b_hh"])
    wihAT = np.ascontiguousarray(wihT_d[:EMB])
    wihQT = np.ascontiguousarray(wihT_d[EMB:])

    q_idx = np.zeros(13 * 128, np.int32)
    q_idx[:B * LQ] = inp["question"].T.reshape(-1).astype(np.int32)
    a_idx = inp["answer"][:, :STEPS].T.reshape(-1).astype(np.int32)

    lin_w = inp["lin_w"].astype(np.float32)   # [32000, 512]
    lin_b = inp["lin_b"].astype(np.float32)

    base = {
        "q_idx": q_idx, "a_idx": a_idx,
        "q_emb": inp["q_emb_w"].astype(f16),
        "a_emb": inp["a_emb_w"].astype(f16),
        "w_ihT_enc": wihT_e, "w_hhT_enc": whhT_e, "bias_enc": b_e,
        "w_ihAT": wihAT, "w_ihQT": wihQT, "w_hhT_dec": whhT_d,
        "bias_dec": b_d,
        "q_lin_wT": np.ascontiguousarray(inp["q_lin_w"].T).astype(f16),
        "q_lin_b": inp["q_lin_b"][None, :].astype(f16),
        "i128f": np.eye(128, dtype=np.float32),
        "i128s": (16.0 * np.eye(128)).astype(f16),
        "i128h": np.eye(128, dtype=f16),
        "i32h": np.eye(32, dtype=f16),
        "ones1": np.ones((1, 32), f16),
    }
    in_maps = []
    for core in range(NCORES):
        m = dict(base)
        sl = lin_w[VSH * core: VSH * (core + 1)]          # [4000, 512]
        slp = np.zeros((VPAD, HID), np.float32)
        slp[:VSH] = sl
        wT = 16.0 * slp.T                                  # [512, VPAD] x16
        arr = wT.reshape(2, 2, 128, VPAD).transpose(2, 0, 1, 3)
        m["lin_w8"] = np.ascontiguousarray(arr.reshape(128, -1)).astype(E4)
        bp = np.zeros(VPAD, np.float32)
        bp[:VSH] = lin_b[VSH * core: VSH * (core + 1)]
        m["lin_b"] = np.ascontiguousarray(bp.reshape(32, 128).T)
        in_maps.append(m)

    _cache["in_maps"] = in_maps
    res = run_bass_kernel_spmd(nc, in_maps, core_ids=list(range(NCORES)))
    _cache["last_res"] = res
    out = np.concatenate(
        [res.results[i]["out"][:, :VSH, :] for i in range(NCORES)], axis=1)
    return out.astype(np.float32)


if __name__ == "__main__":
    import reference
    ins = reference.setup_inputs()
    ref = np.asarray(reference.reference(**ins))
    got = kernel(**{k: np.asarray(v) for k, v in ins.items()})
    err = np.abs(got - ref).max() / (np.abs(ref).max() + 1e-12)
    print("max abs err:", np.abs(got - ref).max(), "rel:", err)


def run_traced():
    nc = _cache["prog"]
    return run_bass_kernel_spmd(nc, _cache["in_maps"],
                                core_ids=list(range(NCORES)), trace=True)



# revision 3
# speedup vs baseline: 2.0128x; 2.0128x over previous
"""Trainium2 Bass kernel for an LSTM encoder-decoder chatbot model.

Model: question -> embed -> LSTM(512) -> linear(256) = q_out
       answer[:, :256] -> embed -> concat(q_out) -> LSTM(512) -> linear(32000)
Output: logits [B=32, W=32000, STEPS=256] f32.

Sharding: all 8 cores run the full (replicated) encoder + decoder
recurrence; the dominant 512x32000 output projection is sharded
column-wise (vocab) across cores; each core emits 4000 vocab rows.

Recurrence layout ("T-layout"): gate units live on PSUM partitions and
batch on the free dim, so each step's hidden state is produced already
transposed for the next step's matmul (no per-step transpose). The
h @ w_hh matmuls run fp8e4m3 + DoubleRow (h stored as 16*h fp8); the
x-side (embeddings, w_ih, qb) stays fp16 for accuracy. All gate
pre-activations accumulate in PSUM at 256x scale; activations apply
scale=1/256. Cell update runs in fp16 on DVE with a fused
scalar_tensor_tensor producing 16*h fp8 directly.

Projection: fp8 DoubleRow as 16*h @ 16*w -> PSUM (256x logits), copied
to fp16 SBUF (DVE/Pool alternating) and DMA'd out raw; the host divides
by 256 and adds lin_b.
"""
import sys
import numpy as np

sys.path.insert(0, '/opt/trn_rl_repo')

import concourse.bass as bass  # noqa: E402
import concourse.bacc as bacc  # noqa: E402
import concourse.mybir as mybir  # noqa: E402
import concourse.tile as tile  # noqa: E402
from concourse.bass import IndirectOffsetOnAxis  # noqa: E402
from concourse.bass_utils import run_bass_kernel_spmd  # noqa: E402

F32 = mybir.dt.float32
F16 = mybir.dt.float16
F8 = mybir.dt.float8e4
I32DT = mybir.dt.int32
AF = mybir.ActivationFunctionType
DR = mybir.MatmulPerfMode.DoubleRow
MUL = mybir.AluOpType.mult

W_VOCAB = 32000
EMB = 256
STEPS = 256
HID = 512
QOUT = 256
B = 32
LQ = 50
NCORES = 8
VSH = W_VOCAB // NCORES      # 4000 vocab rows per core
VPAD = 4096                   # padded to 32 tiles of 128
G = 4 * HID                   # 2048 gate units
TBLK = 32                     # decoder steps per hs block (8 blocks)
NBLK = STEPS // TBLK
QTILES = 13                   # ceil(B*LQ/128) question token tiles
ATILES = 64                   # B*STEPS/128 answer token tiles

_cache = {}

# Gate-unit ordering: col block bb in 0..15 covers 32 batch cols; gate
# type gt = bb//4 with order [i, f, o, g] (sigmoid block contiguous),
# k = bb%4 the 128-unit chunk. Unit (bb, u) maps to original gate row
# ghat = orig[gt]*512 + k*128 + u with PyTorch order i,f,g,o.
_ORIG = np.array([0, 1, 3, 2])


def _ghat_order():
    bb = np.arange(G) // 128
    u = np.arange(G) % 128
    return _ORIG[bb // 4] * HID + (bb % 4) * 128 + u


def build_program():
    nc = bacc.Bacc("TRN2", target_bir_lowering=False, debug=False,
                   num_devices=NCORES)

    def inp(name, shape, dt):
        return nc.dram_tensor(name, shape, dt, kind="ExternalInput").ap()

    q_idx = inp("q_idx", [QTILES * 128], I32DT)        # padded 1664, t-major
    a_idx = inp("a_idx", [STEPS * B], I32DT)           # 8192, t-major
    q_emb16 = inp("q_emb16", [W_VOCAB, EMB], F16)      # 16x
    a_emb16 = inp("a_emb16", [W_VOCAB, EMB], F16)      # 16x
    wih_e16 = inp("wih_e16", [128, 2 * G], F16)        # T-packed, 16x
    whh_e8 = inp("whh_e8", [128, 4 * G], F8)           # DR-packed, 16x
    bias_e16 = inp("bias_e16", [128, 512], F16)        # 256x, b-bcast
    wihA16 = inp("wihA16", [128, 2 * G], F16)
    wihQ16 = inp("wihQ16", [128, 2 * G], F16)
    whh_d8 = inp("whh_d8", [128, 4 * G], F8)
    bias_d16 = inp("bias_d16", [128, 512], F16)
    qlw16 = inp("qlw16", [128, 1024], F16)             # 16x
    qlb16 = inp("qlb16", [128, 2], F32)                # 16x
    lin_w8 = inp("lin_w8", [128, 2 * 2 * VPAD], F8)    # fp8 pairs, 16x
    i128h = inp("i128h", [128, 128], F16)
    out2 = nc.dram_tensor("out2", [NBLK, 2, VPAD // 128, 128, 512], F16,
                          kind="ExternalOutput").ap()

    with tile.TileContext(nc) as tc:
        _build(nc, tc, locals())
    nc.compile()
    return nc


def _build(nc, tc, t):
    from contextlib import ExitStack
    ctx = ExitStack()
    with ctx:
        _build_inner(nc, tc, t, ctx)


def _build_inner(nc, tc, t, ctx):
    # ---- pools -------------------------------------------------------
    wpool = ctx.enter_context(tc.tile_pool(name="weights", bufs=1))
    const = ctx.enter_context(tc.tile_pool(name="const", bufs=1))
    embp = ctx.enter_context(tc.tile_pool(name="embp", bufs=4))
    seqp = ctx.enter_context(tc.tile_pool(name="seqp", bufs=1))
    state = ctx.enter_context(tc.tile_pool(name="state", bufs=4))
    ew = ctx.enter_context(tc.tile_pool(name="ew", bufs=4))
    hsp = ctx.enter_context(tc.tile_pool(name="hsp", bufs=3))
    outp = ctx.enter_context(tc.tile_pool(name="outp", bufs=6))
    ps_g = ctx.enter_context(tc.tile_pool(name="ps_g", bufs=2, space="PSUM"))
    ps_p = ctx.enter_context(tc.tile_pool(name="ps_p", bufs=4, space="PSUM"))
    ps_tr = ctx.enter_context(tc.tile_pool(name="ps_tr", bufs=2, space="PSUM"))

    def load(pool, ap, name):
        s = pool.tile(list(ap.shape), ap.dtype, tag=name, name=name)
        nc.sync.dma_start(s[:], ap[:])
        return s

    # ---- resident weights/constants ---------------------------------
    wih_e = load(wpool, t["wih_e16"], "wih_e")
    whh_e = load(wpool, t["whh_e8"], "whh_e")
    b_e = load(const, t["bias_e16"], "b_e")
    wihA = load(wpool, t["wihA16"], "wihA")
    wihQ = load(wpool, t["wihQ16"], "wihQ")
    whh_d = load(wpool, t["whh_d8"], "whh_d")
    b_d = load(const, t["bias_d16"], "b_d")
    qlw = load(wpool, t["qlw16"], "qlw")
    qlb = load(const, t["qlb16"], "qlb")
    linw8 = wpool.tile([128, 2, 2, VPAD], F8, tag="linw8", name="linw8")
    nc.sync.dma_start(linw8[:].rearrange("p a b c -> p (a b c)"),
                      t["lin_w8"][:])
    I128h = load(const, t["i128h"], "I128h")

    qidx_sb = load(const, t["q_idx"].rearrange("(n p) -> n p", p=128)
                   .rearrange("n p -> p n"), "qidx")   # [128, 13]
    aidx_sb = load(const, t["a_idx"].rearrange("(n p) -> n p", p=128)
                   .rearrange("n p -> p n"), "aidx")   # [128, 64]

    # ---- embedding gather + transpose into xT [128, 2*ntiles*128] ----
    qT = seqp.tile([128, 2 * QTILES * 128], F16, tag="qT", name="qT")
    aT = seqp.tile([128, 2 * ATILES * 128], F16, tag="aT", name="aT")

    def emit_xT(table, idx_sb, i, dstT, stride):
        rows = embp.tile([128, EMB], F16, tag="gather", name="grows")
        nc.gpsimd.indirect_dma_start(
            out=rows[:], out_offset=None, in_=table[:],
            in_offset=IndirectOffsetOnAxis(ap=idx_sb[:, i:i + 1], axis=0))
        for kk in range(2):
            p = ps_tr.tile([128, 128], F16, space="PSUM", tag="tr",
                           name="trp")
            nc.tensor.transpose(p[:], rows[:, 128 * kk:128 * (kk + 1)],
                                I128h[:])
            nc.vector.tensor_copy(
                dstT[:, stride * kk + 128 * i: stride * kk + 128 * (i + 1)],
                p[:])

    for i in range(QTILES):
        emit_xT(t["q_emb16"], qidx_sb, i, qT, QTILES * 128)

    # ---- one LSTM step in T-layout -----------------------------------
    def step(tt, h8, c, xT, xstride, wih16, whh8, const16, want_hs):
        gp = ps_g.tile([128, 512], F32, space="PSUM", tag="gates")
        nc.tensor.matmul(gp[:], I128h[:], const16[:], start=True, stop=False)
        for kk in range(2):
            rhs = xT[:, xstride * kk + 32 * tt: xstride * kk + 32 * tt + 32]
            for bb in range(16):
                nc.tensor.matmul(
                    gp[:, 32 * bb:32 * (bb + 1)],
                    wih16[:, G * kk + 128 * bb: G * kk + 128 * (bb + 1)],
                    rhs, start=False, stop=(h8 is None and kk == 1))
        if h8 is not None:
            h8v = h8[:].rearrange("p (j pr b) -> p j pr b", j=2, pr=2)
            whhv = whh8[:].rearrange("p (j pr c) -> p j pr c", j=2, pr=2)
            for j in range(2):
                for bb in range(16):
                    nc.tensor.matmul(
                        gp[:, 32 * bb:32 * (bb + 1)],
                        whhv[:, j, :, 128 * bb:128 * (bb + 1)],
                        h8v[:, j], start=False, stop=(j == 1), perf_mode=DR)
        sg = ew.tile([128, 512], F16, tag="sg")
        nc.scalar.activation(sg[:, 0:384], gp[:, 0:384], AF.Sigmoid,
                             scale=1.0 / 256.0)
        nc.scalar.activation(sg[:, 384:512], gp[:, 384:512], AF.Tanh,
                             scale=1.0 / 256.0)
        c_new = state.tile([128, 128], F16, tag="c")
        if c is None:
            nc.vector.tensor_mul(c_new[:], sg[:, 0:128], sg[:, 384:512])
        else:
            fc = ew.tile([128, 128], F16, tag="fc")
            nc.vector.tensor_mul(fc[:], sg[:, 128:256], c[:])
            igg = ew.tile([128, 128], F16, tag="igg")
            nc.vector.tensor_mul(igg[:], sg[:, 0:128], sg[:, 384:512])
            nc.vector.tensor_add(c_new[:], igg[:], fc[:])
        th = ew.tile([128, 128], F16, tag="th")
        nc.scalar.activation(th[:], c_new[:], AF.Tanh)
        h8_new = state.tile([128, 128], F8, tag="h8")
        nc.vector.scalar_tensor_tensor(h8_new[:], sg[:, 256:384], 16.0,
                                       th[:], op0=MUL, op1=MUL)
        return h8_new, c_new

    # ---- encoder (aT emission interleaved) ---------------------------
    h8 = None
    c = None
    a_emitted = 0
    for tt in range(LQ):
        h8, c = step(tt, h8, c, qT, QTILES * 128, wih_e, whh_e, b_e, False)
        want = (tt + 1) * ATILES // LQ
        while a_emitted < want:
            emit_xT(t["a_emb16"], aidx_sb, a_emitted, aT, ATILES * 128)
            a_emitted += 1
    while a_emitted < ATILES:
        emit_xT(t["a_emb16"], aidx_sb, a_emitted, aT, ATILES * 128)
        a_emitted += 1

    # ---- q_out (transposed) and qb = q_out @ wihQ.T + bias_dec -------
    h16 = seqp.tile([128, 128], F16, tag="h16", name="h16")
    nc.vector.tensor_copy(h16[:], h8[:])
    qoT = seqp.tile([128, 64], F16, tag="qoT", name="qoT")
    for qt in range(2):
        qp = ps_p.tile([128, 32], F32, space="PSUM", tag="proj", name="qp")
        for k in range(4):
            nc.tensor.matmul(qp[:],
                             qlw[:, 256 * k + 128 * qt:256 * k + 128 * (qt + 1)],
                             h16[:, 32 * k:32 * (k + 1)],
                             start=(k == 0), stop=(k == 3))
        nc.scalar.activation(qoT[:, 32 * qt:32 * (qt + 1)], qp[:],
                             AF.Identity, scale=1.0 / 16.0,
                             bias=qlb[:, qt:qt + 1])
    qb16 = seqp.tile([128, 512], F16, tag="qb16", name="qb16")
    qbp = ps_p.tile([128, 512], F32, space="PSUM", tag="proj", name="qbp")
    nc.tensor.matmul(qbp[:], I128h[:], b_d[:], start=True, stop=False)
    for qq in range(2):
        rhs = qoT[:, 32 * qq:32 * (qq + 1)]
        for bb in range(16):
            nc.tensor.matmul(
                qbp[:, 32 * bb:32 * (bb + 1)],
                wihQ[:, G * qq + 128 * bb: G * qq + 128 * (bb + 1)],
                rhs, start=False, stop=(qq == 1))
    nc.scalar.activation(qb16[:], qbp[:], AF.Identity)

    # ---- decoder + projection, software-pipelined --------------------
    out2 = t["out2"]

    def proj_m(hs_b, blk, m):
        hsk = hs_b[:].rearrange("p (k bt) -> p k bt", k=4)
        for s in range(2):
            pp = ps_p.tile([128, 512], F32, space="PSUM", tag="proj")
            for j in range(2):
                nc.tensor.matmul(
                    pp[:], linw8[:, j, :, 128 * m:128 * (m + 1)],
                    hsk[:, 2 * j:2 * j + 2, 512 * s:512 * (s + 1)],
                    start=(j == 0), stop=(j == 1), perf_mode=DR)
            ot = outp.tile([128, 512], F16, tag="ot")
            nc.vector.tensor_copy(ot[:], pp[:])
            nc.sync.dma_start(out2[blk, s, m], ot[:])

    hs_prev = None
    for blk in range(NBLK):
        hs = hsp.tile([128, 4 * 32 * TBLK], F8, tag="hs", name="hs")
        for dt in range(TBLK):
            tt = blk * TBLK + dt
            h8, c = step(tt, h8, c, aT, ATILES * 128, wihA, whh_d, qb16,
                         True)
            dst = hs[:].rearrange("p (k b t) -> p k b t", k=4, b=32)[:, :, :,
                                                                     dt]
            nc.gpsimd.tensor_copy(dst, h8[:].rearrange("p (k b) -> p k b",
                                                       k=4))
            if hs_prev is not None:
                proj_m(hs_prev, blk - 1, dt)
        hs_prev = hs
    for m in range(VPAD // 128):
        proj_m(hs_prev, NBLK - 1, m)


# ---- host-side packing ----------------------------------------------

def _pack_wih(w):
    """w [2048, K] (K mult of 128) -> [128, (K//128)*2048] f16, 16x,
    T-layout: out[p, kk*2048 + bb*128 + u] = 16*w[ghat(bb,u), kk*128+p]."""
    K = w.shape[1]
    Wp = (16.0 * w)[_ghat_order()]                    # [2048, K]
    arr = Wp.reshape(16, 128, K // 128, 128).transpose(3, 2, 0, 1)
    return np.ascontiguousarray(arr.reshape(128, -1)).astype(np.float16)


def _pack_whh8(w):
    """w [2048, 512] -> [128, 8192] fp8 DR-packed, 16x:
    out[p, j*4096 + pair*2048 + bb*128 + u] = 16*w[ghat, (2j+pair)*128+p]."""
    import ml_dtypes
    Wp = (16.0 * w)[_ghat_order()]                    # [2048, 512]
    arr = Wp.reshape(16, 128, 2, 2, 128).transpose(4, 2, 3, 0, 1)
    return np.ascontiguousarray(arr.reshape(128, -1)).astype(
        ml_dtypes.float8_e4m3)


def _pack_bias(b):
    """b [2048] -> [128, 512] f16 at 256x, broadcast over batch cols."""
    bp = (256.0 * b)[_ghat_order()].reshape(16, 128)   # [bb, u]
    arr = np.broadcast_to(bp.T[:, :, None], (128, 16, 32))
    return np.ascontiguousarray(arr.reshape(128, 512)).astype(np.float16)


def kernel(**inputs):
    import ml_dtypes
    E4 = ml_dtypes.float8_e4m3
    inp = {k: np.asarray(v) for k, v in inputs.items()}
    if "prog" not in _cache:
        _cache["prog"] = build_program()
    nc = _cache["prog"]

    f16 = np.float16
    f32 = np.float32

    wih_d = inp["a_lstm_w_ih"].astype(f32)
    qlw = inp["q_lin_w"].astype(f32)                   # [256, 512]
    qlw_packed = (16.0 * qlw).reshape(2, 128, 4, 128).transpose(3, 2, 0, 1)

    q_idx = np.zeros(QTILES * 128, np.int32)
    q_idx[:B * LQ] = inp["question"].T.reshape(-1).astype(np.int32)
    a_idx = inp["answer"][:, :STEPS].T.reshape(-1).astype(np.int32)

    lin_w = inp["lin_w"].astype(f32)                   # [32000, 512]
    lin_b = inp["lin_b"].astype(f32)

    base = {
        "q_idx": q_idx, "a_idx": a_idx,
        "q_emb16": (16.0 * inp["q_emb_w"].astype(f32)).astype(f16),
        "a_emb16": (16.0 * inp["a_emb_w"].astype(f32)).astype(f16),
        "wih_e16": _pack_wih(inp["q_lstm_w_ih"].astype(f32)),
        "whh_e8": _pack_whh8(inp["q_lstm_w_hh"].astype(f32)),
        "bias_e16": _pack_bias((inp["q_lstm_b_ih"] +
                                inp["q_lstm_b_hh"]).astype(f32)),
        "wihA16": _pack_wih(wih_d[:, :EMB]),
        "wihQ16": _pack_wih(wih_d[:, EMB:]),
        "whh_d8": _pack_whh8(inp["a_lstm_w_hh"].astype(f32)),
        "bias_d16": _pack_bias((inp["a_lstm_b_ih"] +
                                inp["a_lstm_b_hh"]).astype(f32)),
        "qlw16": np.ascontiguousarray(
            qlw_packed.reshape(128, 1024)).astype(f16),
        "qlb16": np.ascontiguousarray(
            (16.0 * inp["q_lin_b"].astype(f32)).reshape(2, 128).T
        ).astype(f32),
        "i128h": np.eye(128, dtype=f16),
    }
    in_maps = []
    for core in range(NCORES):
        m = dict(base)
        sl = lin_w[VSH * core: VSH * (core + 1)]       # [4000, 512]
        slp = np.zeros((VPAD, HID), f32)
        slp[:VSH] = sl
        wT = 16.0 * slp.T                              # [512, VPAD] x16
        arr = wT.reshape(2, 2, 128, VPAD).transpose(2, 0, 1, 3)
        m["lin_w8"] = np.ascontiguousarray(arr.reshape(128, -1)).astype(E4)
        in_maps.append(m)

    _cache["in_maps"] = in_maps
    res = run_bass_kernel_spmd(nc, in_maps, core_ids=list(range(NCORES)))
    _cache["last_res"] = res
    outs = []
    for i in range(NCORES):
        arr = np.asarray(res.results[i]["out2"], dtype=f32)
        arr = arr.reshape(NBLK, 2, VPAD // 128, 128, 16, 32)
        arr = arr.transpose(1, 4, 2, 3, 0, 5).reshape(B, VPAD, STEPS)
        outs.append(arr[:, :VSH, :])
    out = np.concatenate(outs, axis=1) * (1.0 / 256.0)
    out += lin_b[None, :, None]
    return out.astype(f32)


if __name__ == "__main__":
    import reference
    ins = reference.setup_inputs()
    ref = np.asarray(reference.reference(**ins))
    got = kernel(**{k: np.asarray(v) for k, v in ins.items()})
    err = np.abs(got - ref).max() / (np.abs(ref).max() + 1e-12)
    print("max abs err:", np.abs(got - ref).max(), "rel:", err)


def run_traced():
    nc = _cache["prog"]
    return run_bass_kernel_spmd(nc, _cache["in_maps"],
                                core_ids=list(range(NCORES)), trace=True)
